# revision 31
# baseline (speedup 1.0000x reference)
"""BiConv GNN kernel v2 for 8 Trainium2 NeuronCores: gather + PE segment-sum.

out  = relu(norm   * (x + scatter_add(x[sources] -> targets)) @ W_out.T)
back = relu(norm_t * (x + scatter_add(x[targets] -> sources)) @ W_back.T)
result = out + back

Strategy (collective-free, node-output-sharded, NO dma_scatter_add):
  * Each of the 8 cores owns 25000 output nodes = 196 groups of 128 dsts,
    grouped into blocks of 16 groups.
  * Per direction, edges are routed (host) to the dst-owning core and
    sorted by (block, src chunk of 65536 [signed-int16 gather window
    around a midpoint-biased base], group).
  * The core dma_gathers x[src] rows in that order (2048-idx instructions
    round-robined over 4 SWDGE queues: descriptor generation parallelizes
    across queues -- ~2.5ns/idx vs 8.3ns on one queue, hw-measured).
  * Each 128-edge slab of a gather tile is segment-summed into its
    group's PSUM accumulator via a one-hot Sel matmul (Sel[edge, dstslot]
    generated on DVE as iota==id; id=999 rows vanish, so index-stream
    padding and cross-core structure padding contribute exact zeros).
  * Slabs that straddle a group boundary issue one masked matmul per
    group; PSUM accumulation uses start/stop flags per (block, group).
  * Final phase fused per block, entirely on-chip: agg(+x)*norm,
    PE-transpose, 64x64 matmul, ReLU, add both directions, transpose
    back, contiguous DMA out. The OUT direction parks its aggs in SBUF
    ([128, 196, 64] tile) so no HBM roundtrip for aggregates at all.
"""

import sys

sys.path.insert(0, "/opt/trn_rl_repo")

import numpy as np

import concourse.bass as bass  # noqa: F401
import concourse.mybir as mybir
from concourse import bacc
from concourse.tile import TileContext

F32 = mybir.dt.float32
BF16 = mybir.dt.bfloat16
I16 = mybir.dt.int16

NCORES = 8


class Cfg:
    def __init__(self, n=200000, e=3200000, c=64, chunk=65536, tile_e=2048,
                 gpb=16, nq=4, idxcap=8192, selstage=64, selbatch=16):
        self.N = n
        self.E = e
        self.C = c
        # 64K-row gather windows: int16 gather indices are SIGN-EXTENDED by
        # the Q7 ucode and mid-stream negatives address below the AP base,
        # so biasing the base to the window middle doubles the window to
        # 65536 rows (only TRAILING negative idxs get trimmed -- the host
        # tail-swap in _prep_direction guarantees none).
        self.CHUNK = chunk
        self.TILE_E = tile_e          # idx per gather instruction
        self.GPB = gpb                # groups (of 128 dsts) per block
        self.NQ = nq                  # SWDGE queues for gathers
        self.IDXCAP = idxcap          # max idx per (block, chunk) run
        self.SELSTAGE = selstage      # sel-id columns per staged tile
        self.SELBATCH = selbatch      # sel matrices per DVE gen op
        self.SLICE = n // NCORES      # 25000
        self.NGROUP = -(-self.SLICE // 128)       # 196
        self.SP = self.NGROUP * 128               # 25088
        self.NBLOCK = -(-self.NGROUP // gpb)      # 13
        self.NCHUNK = -(-n // chunk)              # 4

    def chunk_bias(self, c):
        """Gather AP base row for chunk c (window = bias +/- 32768).
        64K windows get the midpoint bias; 32K windows and the tail
        window are positive-only (rows all fit above the base)."""
        lo = c * self.CHUNK
        if self.CHUNK > 32768 and lo + self.CHUNK <= self.N:
            return lo + 32768
        return lo

    def chunk_csize(self, c):
        b = self.chunk_bias(c)
        return min(32768, self.N - b)


def _round_up(v, m):
    return -(-v // m) * m


def _pack_idx(arr):
    """[16, n//16] int16 layout (edge i at [i%16, i//16]), replicated x8."""
    assert arr.shape[0] % 16 == 0
    p16 = np.ascontiguousarray(arr.astype(np.int16).reshape(-1, 16).T)
    return np.ascontiguousarray(np.tile(p16, (8, 1)))


def _prep_direction(src_all, dst_all, cfg):
    """Sort/bucket one direction's edges; build the shared (cross-core)
    instruction structure and per-core index/sel streams.

    Returns (plan, gidx[8], sel[8]).
      plan.runs: [(blk, chk, n_pad, idx_off, instrs, slabrecs)]
        instrs:   [(off_in_run, ni)]
        slabrecs: [(slab_idx_in_run, [(k_local, mm_idx), ...])]
      plan.first/last: record flags {mm_idx: bool}
      plan.memsets: per block, list of k_local with no records at all
    """
    NB, NCH, GPB = cfg.NBLOCK, cfg.NCHUNK, cfg.GPB

    cores = []
    for j in range(NCORES):
        m = (dst_all // cfg.SLICE) == j
        s = src_all[m]
        d = dst_all[m] - j * cfg.SLICE
        grp = d >> 7
        chk = s // cfg.CHUNK
        blk = grp // GPB
        # secondary src key: within each group segment the gather
        # descriptors read ascending addresses (better HBM channel/bank
        # behavior for the SDMA drain); order within a group is free
        o = np.lexsort((s, grp, chk, blk))
        cores.append((s[o], d[o], grp[o], chk[o], blk[o]))

    lens = np.zeros((NCORES, NB, NCH), dtype=np.int64)
    starts = np.zeros((NCORES, NB, NCH), dtype=np.int64)
    for j in range(NCORES):
        _, _, grp, chk, blk = cores[j]
        key = (blk * NCH + chk).astype(np.int64)
        cnt = np.bincount(key, minlength=NB * NCH)
        lens[j] = cnt.reshape(NB, NCH)
        flat = lens[j].reshape(-1)
        starts[j] = np.concatenate([[0], np.cumsum(flat)[:-1]]).reshape(NB, NCH)

    runs = []
    gidx_stream = [[] for _ in range(NCORES)]
    selcols = [[] for _ in range(NCORES)]
    rec_bk = []
    idx_off = 0

    for b in range(NB):
        GB = GPB if b < NB - 1 else cfg.NGROUP - (NB - 1) * GPB
        for c in range(NCH):
            n_max = int(lens[:, b, c].max())
            if n_max == 0:
                continue
            # split oversized runs so idx tiles stay bounded
            sub_bounds = []
            o = 0
            while o < n_max:
                take = min(cfg.IDXCAP, n_max - o)
                sub_bounds.append((o, take))
                o += take
            for (sub_o, sub_n) in sub_bounds:
                n_pad = _round_up(sub_n, 128)
                run_g = np.zeros((NCORES, n_pad), dtype=np.int64)
                run_k = np.full((NCORES, n_pad), -1, dtype=np.int64)
                run_doff = np.zeros((NCORES, n_pad), dtype=np.int64)
                for j in range(NCORES):
                    s_, d_, grp_, _, _ = cores[j]
                    a = starts[j, b, c]
                    L = int(lens[j, b, c])
                    lo = min(sub_o, L)
                    hi = min(sub_o + sub_n, L)
                    n = hi - lo
                    if n > 0:
                        run_g[j, :n] = s_[a + lo : a + hi] - cfg.chunk_bias(c)
                        run_k[j, :n] = grp_[a + lo : a + hi] - b * GPB
                        run_doff[j, :n] = d_[a + lo : a + hi] & 127
                instrs = []
                o2 = 0
                while o2 < n_pad:
                    ni = min(cfg.TILE_E, n_pad - o2)
                    instrs.append((o2, ni))
                    o2 += ni
                # The gather ucode TRIMS trailing negative idxs per
                # instruction, dropping real edges.  Guarantee every
                # instruction-tail position holds a non-negative idx by
                # swapping it with a same-group non-negative edge (the
                # swap moves idx+doff together and stays within the same
                # group, so slab records are unchanged).
                tails = set(o + ni - 1 for (o, ni) in instrs)
                for j in range(NCORES):
                    for P in sorted(tails):
                        if run_g[j, P] >= 0:
                            continue
                        k = run_k[j, P]
                        cand = np.nonzero(
                            (run_k[j] == k) & (run_g[j] >= 0)
                        )[0]
                        cand = [q for q in cand if q not in tails]
                        if cand:
                            # same-group swap: records unchanged
                            q = cand[0]
                            swap = (run_g, run_doff)
                        else:
                            # cross-group (or padding) swap: move the whole
                            # edge record; costs at most one extra sel record
                            cand = np.nonzero(run_g[j] >= 0)[0]
                            cand = [q for q in cand if q not in tails]
                            assert cand, "run has no non-negative position"
                            q = cand[0]
                            swap = (run_g, run_doff, run_k)
                        for arr in swap:
                            arr[j, P], arr[j, q] = arr[j, q], arr[j, P]
                n_slab = n_pad // 128
                rk = run_k.reshape(NCORES, n_slab, 128)
                rd = run_doff.reshape(NCORES, n_slab, 128)
                slabrecs = []
                for sl in range(n_slab):
                    ks = np.unique(rk[:, sl, :])
                    ks = ks[ks >= 0]
                    if len(ks) == 0:
                        continue
                    recs = []
                    for k in ks:
                        mm = len(rec_bk)
                        rec_bk.append((b, int(k)))
                        for j in range(NCORES):
                            col = np.where(
                                rk[j, sl] == k, rd[j, sl], 999
                            ).astype(np.float32)
                            selcols[j].append(col)
                        recs.append((int(k), mm))
                    slabrecs.append((sl, recs))
                for j in range(NCORES):
                    gidx_stream[j].append(run_g[j])
                runs.append((b, c, n_pad, idx_off, instrs, slabrecs))
                idx_off += n_pad

    T = idx_off
    M = len(rec_bk)
    Mpad = max(_round_up(M, cfg.SELSTAGE), cfg.SELSTAGE)

    first = {}
    last = {}
    for i, bk in enumerate(rec_bk):
        if bk not in first:
            first[bk] = i
        last[bk] = i
    first_set = set(first.values())
    last_set = set(last.values())

    memsets = [[] for _ in range(NB)]
    for b in range(NB):
        GB = GPB if b < NB - 1 else cfg.NGROUP - (NB - 1) * GPB
        for k in range(GB):
            if (b, k) not in first:
                memsets[b].append(k)

    import ml_dtypes

    gidx = [_pack_idx(np.concatenate(gidx_stream[j])) for j in range(NCORES)]
    sel = []
    for j in range(NCORES):
        # ids stored PAIR-DUPLICATED ([128, Mpad, 2], both lanes equal): the
        # DVE is_equal then reads the ids operand as a step-1 packed bf16
        # pair (stride-0 only on the dim above), which keeps the 2x_1P
        # packed mode available instead of degrading to 1x on a stride-0
        # innermost broadcast.
        s = np.full((128, Mpad, 2), 999.0, dtype=np.float32)
        if M:
            col = np.stack(selcols[j], axis=1)
            s[:, :M, 0] = col
            s[:, :M, 1] = col
        sel.append(np.ascontiguousarray(s.astype(ml_dtypes.bfloat16)))

    plan = dict(
        runs=runs, first=first_set, last=last_set, memsets=memsets,
        T=T, M=M, Mpad=Mpad,
    )
    return plan, gidx, sel


def _build_graph(cfg, plan_o, plan_b):
    C = cfg.C
    GPB = cfg.GPB
    IS_EQ = mybir.AluOpType.is_equal
    SB = cfg.SELBATCH
    SS = cfg.SELSTAGE

    nc = bacc.Bacc(None, target_bir_lowering=False, num_swdge_queues=cfg.NQ)

    x_full = nc.declare_dram_parameter("x_full", [cfg.N, C], F32, False)
    x_sl = nc.declare_dram_parameter("x_sl", [cfg.SP, C], F32, False)
    norm_sl = nc.declare_dram_parameter("norm_sl", [cfg.SP, 1], F32, False)
    normt_sl = nc.declare_dram_parameter("normt_sl", [cfg.SP, 1], F32, False)
    wot = nc.declare_dram_parameter("wot", [C, C], F32, False)
    wbt = nc.declare_dram_parameter("wbt", [C, C], F32, False)
    ident = nc.declare_dram_parameter("ident", [128, 128], F32, False)
    iota8 = nc.declare_dram_parameter("iota8", [128, SB, 128], BF16, False)
    gidx_o = nc.declare_dram_parameter(
        "gidx_o", [128, plan_o["T"] // 16], I16, False)
    gidx_b = nc.declare_dram_parameter(
        "gidx_b", [128, plan_b["T"] // 16], I16, False)
    sel_o = nc.declare_dram_parameter(
        "sel_o", [128, plan_o["Mpad"], 2], BF16, False)
    sel_b = nc.declare_dram_parameter(
        "sel_b", [128, plan_b["Mpad"], 2], BF16, False)
    out = nc.declare_dram_parameter("out", [cfg.SP, C], F32, True)

    qctr = [0]

    with TileContext(nc) as tc:
        with (
            tc.tile_pool(name="const", bufs=1) as cpool,
            tc.tile_pool(name="idx", bufs=4) as ipool,
            tc.tile_pool(name="gt", bufs=10) as gpool,
            tc.tile_pool(name="gt16", bufs=10) as g16pool,
            tc.tile_pool(name="selid", bufs=3) as sidpool,
            tc.tile_pool(name="sel", bufs=6) as selpool,
            tc.tile_pool(name="fin", bufs=2) as fpool,
            tc.tile_pool(name="fblk", bufs=3) as fbpool,
            tc.tile_pool(name="fy", bufs=2) as fypool,
            # pacc x2 overlaps block b+1 accumulation with block b's
            # finish phase (block transitions stalled all 4 gather queues
            # ~25-45us each, 26 times).  pt/pf at 1 buf to fit the 16KB
            # PSUM budget: 8 + 2 + 4 + 2.
            tc.tile_pool(name="pacc", bufs=2, space="PSUM") as paccpool,
            tc.tile_pool(name="pt", bufs=1, space="PSUM") as ptpool,
            tc.tile_pool(name="pm", bufs=2, space="PSUM") as pmpool,
            tc.tile_pool(name="pf", bufs=1, space="PSUM") as pfpool,
        ):
            wot_s = cpool.tile([C, C], F32, tag="wot")
            wbt_s = cpool.tile([C, C], F32, tag="wbt")
            id_s = cpool.tile([128, 128], F32, tag="ident")
            iota_s = cpool.tile([128, SB, 128], BF16, tag="iota8")
            agg_o = cpool.tile([128, cfg.NGROUP, C], F32, tag="agg_o")
            # const loads go on the ACT HWDGE ring so the first gather's
            # gix DMA (SP ring) isn't stuck behind them in FIFO order
            nc.scalar.dma_start(out=wot_s[:], in_=wot[:])
            nc.scalar.dma_start(out=wbt_s[:], in_=wbt[:])
            nc.scalar.dma_start(out=id_s[:], in_=ident[:])
            nc.scalar.dma_start(out=iota_s[:], in_=iota8[:])

            def run_direction(plan, gidx_dram, sel_dram, is_out):
                mm_next = [0]  # next record to prepare (sel staging/gen)
                stage = {"selid": None, "sel": None}

                def prep_record(mm):
                    # stage sel-id columns and generate sel matrices so that
                    # record mm's one-hot matrix is ready in stage["sel"]
                    if mm % SS == 0:
                        st = sidpool.tile([128, SS, 2], BF16, tag="selid")
                        nc.sync.dma_start(
                            out=st[:], in_=sel_dram[:, mm : mm + SS, :])
                        stage["selid"] = st
                    if mm % SB == 0:
                        c0 = mm % SS
                        gen = selpool.tile([128, SB, 128], BF16, tag="sel")
                        # ids as [128, SB, 64, 2]: innermost dim is a real
                        # step-1 bf16 pair (host stores each id twice), the
                        # stride-0 broadcast sits one dim up.
                        ids = (
                            stage["selid"][:, c0 : c0 + SB, :]
                            .unsqueeze(2)
                            .broadcast_to([128, SB, 64, 2])
                        )
                        nc.vector.tensor_tensor(
                            gen[:].rearrange("p s (a b) -> p s a b", b=2),
                            iota_s[:].rearrange(
                                "p s (a b) -> p s a b", b=2),
                            ids,
                            IS_EQ,
                        )
                        stage["sel"] = gen

                cur_blk = [-1]
                pacc_ref = [None]

                def finish_block(b):
                    if b < 0:
                        return
                    GB = (GPB if b < cfg.NBLOCK - 1
                          else cfg.NGROUP - (cfg.NBLOCK - 1) * GPB)
                    pacc = pacc_ref[0]
                    if is_out:
                        # one bulk PSUM->SBUF copy frees pacc in ~1us so the
                        # next block's matmuls don't stall on final-phase reads
                        nc.scalar.copy(
                            agg_o[:, b * GPB : b * GPB + GB, :],
                            pacc[:, 0:GB, :])
                    else:
                        r0 = b * GPB * 128
                        nn = GB * 128
                        agg_b_s = fbpool.tile(
                            [128, GPB, C], F32, tag="agg_b_s")
                        nc.scalar.copy(agg_b_s[:, 0:GB, :], pacc[:, 0:GB, :])
                        x_blk = fbpool.tile([128, GPB, C], F32, tag="x_blk")
                        nrm = fbpool.tile([128, GPB], F32, tag="nrm")
                        nrmt = fbpool.tile([128, GPB], F32, tag="nrmt")
                        o_blk = fbpool.tile([128, GPB, C], F32, tag="o_blk")
                        nc.sync.dma_start(
                            out=x_blk[:, 0:GB, :],
                            in_=x_sl[r0 : r0 + nn, :].rearrange(
                                "(k p) c -> p k c", p=128),
                        )
                        nc.sync.dma_start(
                            out=nrm[:, 0:GB],
                            in_=norm_sl[r0 : r0 + nn, 0].rearrange(
                                "(k p) -> p k", p=128),
                        )
                        nc.sync.dma_start(
                            out=nrmt[:, 0:GB],
                            in_=normt_sl[r0 : r0 + nn, 0].rearrange(
                                "(k p) -> p k", p=128),
                        )
                        # block-batched adds/muls: 4 DVE ops instead of 64
                        # per-group ops (less dispatch + sem churn on DVE)
                        y_ob = fypool.tile([128, GPB, C], F32, tag="y_ob")
                        y_bb = fypool.tile([128, GPB, C], F32, tag="y_bb")
                        nc.vector.tensor_add(
                            y_ob[:, 0:GB, :],
                            agg_o[:, b * GPB : b * GPB + GB, :],
                            x_blk[:, 0:GB, :])
                        nc.vector.tensor_tensor(
                            y_ob[:, 0:GB, :], y_ob[:, 0:GB, :],
                            nrm[:, 0:GB].unsqueeze(2).broadcast_to(
                                [128, GB, C]),
                            mybir.AluOpType.mult)
                        nc.vector.tensor_add(
                            y_bb[:, 0:GB, :], agg_b_s[:, 0:GB, :],
                            x_blk[:, 0:GB, :])
                        nc.vector.tensor_tensor(
                            y_bb[:, 0:GB, :], y_bb[:, 0:GB, :],
                            nrmt[:, 0:GB].unsqueeze(2).broadcast_to(
                                [128, GB, C]),
                            mybir.AluOpType.mult)
                        for k in range(GB):
                            pt = ptpool.tile([C, 256], F32, tag="pt")
                            nc.tensor.transpose(
                                pt[:, 0:128], y_ob[:, k, :], id_s[:])
                            nc.tensor.transpose(
                                pt[:, 128:256], y_bb[:, k, :], id_s[:])
                            yt = fpool.tile([C, 256], F32, tag="yt")
                            nc.scalar.copy(yt[:], pt[:])
                            pm = pmpool.tile([C, 256], F32, tag="pm")
                            nc.tensor.matmul(
                                pm[:, 0:128], wot_s[:], yt[:, 0:128],
                                start=True, stop=True)
                            nc.tensor.matmul(
                                pm[:, 128:256], wbt_s[:], yt[:, 128:256],
                                start=True, stop=True)
                            r = fpool.tile([C, 256], F32, tag="r")
                            nc.scalar.activation(
                                r[:], pm[:], mybir.ActivationFunctionType.Relu)
                            st_ = fpool.tile([C, 128], F32, tag="st")
                            nc.vector.tensor_add(
                                st_[:], r[:, 0:128], r[:, 128:256])
                            pf = pfpool.tile([128, C], F32, tag="pf")
                            nc.tensor.transpose(
                                pf[:], st_[:], id_s[:C, :C])
                            nc.scalar.copy(o_blk[:, k, :], pf[:])
                        # out store on the ACT ring: its o_blk dependency is
                        # produced by ACT copies just before it in the same
                        # stream, so the wait is trivially satisfied and the
                        # SP ring (gix loads) never stalls behind it
                        nc.scalar.dma_start(
                            out=out[r0 : r0 + nn, :].rearrange(
                                "(k p) c -> p k c", p=128),
                            in_=o_blk[:, 0:GB, :],
                        )

                for (b, c, n_pad, idx_off, instrs, slabrecs) in plan["runs"]:
                    if b != cur_blk[0]:
                        finish_block(cur_blk[0])
                        cur_blk[0] = b
                        GB = (GPB if b < cfg.NBLOCK - 1
                              else cfg.NGROUP - (cfg.NBLOCK - 1) * GPB)
                        pacc = paccpool.tile([128, GPB, C], F32, tag="pacc")
                        pacc_ref[0] = pacc
                        # PSUM start=True laziy zeroes the WHOLE 2KB bank, so
                        # interleaved per-group starts would clobber sibling
                        # groups' partials.  Instead: explicit zero once per
                        # block + pure accumulating matmuls (start=False).
                        nc.vector.memset(pacc[:, 0:GB, :], 0.0)
                    pacc = pacc_ref[0]

                    # biased window base: signed idxs reach 32768 rows on
                    # both sides of `base` (see Cfg.chunk_bias)
                    base = cfg.chunk_bias(c)
                    csize = cfg.chunk_csize(c)
                    gix = ipool.tile(
                        [128, cfg.IDXCAP // 16], I16, tag="gix")
                    nc.sync.dma_start(
                        out=gix[:, 0 : n_pad // 16],
                        in_=gidx_dram[:, idx_off // 16 : (idx_off + n_pad) // 16],
                    )
                    gts = []
                    for (o, ni) in instrs:
                        gt = gpool.tile([128, ni // 128, C], F32, tag="gt")
                        nc.gpsimd.dma_gather(
                            gt[:],
                            x_full[base : base + csize, :],
                            gix[:, o // 16 : (o + ni) // 16],
                            ni,
                            ni,
                            C,
                            # single_packet=True (descriptor-packet concat)
                            # crashes the runtime on these 2048-desc gathers
                            # -- keep per-descriptor packets
                            single_packet=False,
                            queue_num=qctr[0] % cfg.NQ,
                        )
                        qctr[0] += 1
                        # PE fp32 matmuls run split+4x-slow; cast the slab
                        # data to bf16 on ACT so slab matmuls run full rate.
                        gt16 = g16pool.tile(
                            [128, ni // 128, C], BF16, tag="gt16")
                        nc.scalar.activation(
                            gt16[:], gt[:], mybir.ActivationFunctionType.Copy)
                        gts.append(gt16)

                    spt = cfg.TILE_E // 128  # slabs per (full) gather tile
                    for (sl, recs) in slabrecs:
                        for (k, mm) in recs:
                            prep_record(mm)
                            sel_t = stage["sel"]
                            gt = gts[sl // spt]
                            col = sl % spt
                            nc.tensor.matmul(
                                pacc[:, k, :],
                                sel_t[:, mm % SB, :],
                                gt[:, col, :],
                                start=False,
                                stop=False,
                                skip_group_check=True,
                            )
                finish_block(cur_blk[0])

            run_direction(plan_o, gidx_o, sel_o, is_out=True)
            run_direction(plan_b, gidx_b, sel_b, is_out=False)

    nc.finalize()
    return nc


def _ensure_ntff_hook():
    """Provide antenv.axon_hooks (missing in this image) so BASS_TRACE=1
    profiling works under axon.  Best-effort; harmless if unavailable."""
    try:
        from antenv import axon_hooks  # noqa: F401

        return
    except ImportError:
        pass
    try:
        import types

        import antenv
        from trn_agent_boot.trn_boot import _ntff_profile_via_ctypes

        mod = types.ModuleType("antenv.axon_hooks")
        state = {"hook": None}
        mod.set_axon_ntff_profile_hook = lambda h: state.__setitem__("hook", h)
        mod.get_axon_ntff_profile_hook = lambda: state["hook"]
        sys.modules["antenv.axon_hooks"] = mod
        antenv.axon_hooks = mod
        hook = _ntff_profile_via_ctypes("/opt/axon/libaxon_pjrt.so")
        if hook is not None:
            mod.set_axon_ntff_profile_hook(hook)
    except Exception:
        pass


def _run(nc, in_maps, core_ids):
    _ensure_ntff_hook()
    from concourse.bass_utils import run_bass_kernel_spmd

    return run_bass_kernel_spmd(nc, in_maps, core_ids)


def _kernel_impl(x, sources, targets, norm, norm_t, W_out, W_back, cfg,
                 run=_run, build_only=False, return_graph=False):
    x = np.ascontiguousarray(x, dtype=np.float32)
    sources = np.asarray(sources, dtype=np.int64)
    targets = np.asarray(targets, dtype=np.int64)
    norm = np.ascontiguousarray(norm, dtype=np.float32)
    norm_t = np.ascontiguousarray(norm_t, dtype=np.float32)

    plan_o, gidx_o, sel_o = _prep_direction(sources, targets, cfg)
    plan_b, gidx_b, sel_b = _prep_direction(targets, sources, cfg)

    nc = _build_graph(cfg, plan_o, plan_b)
    if build_only:
        return None, None

    SP = cfg.SP
    SL = cfg.SLICE
    import ml_dtypes

    ident = np.eye(128, dtype=np.float32)
    iota8 = np.broadcast_to(
        np.arange(128, dtype=np.float32), (128, cfg.SELBATCH, 128)
    ).astype(ml_dtypes.bfloat16)
    wot = np.ascontiguousarray(W_out.T, dtype=np.float32)
    wbt = np.ascontiguousarray(W_back.T, dtype=np.float32)

    def pad_rows(a, rows):
        p = np.zeros((rows, a.shape[1]), dtype=np.float32)
        p[: a.shape[0]] = a
        return p

    in_maps = []
    for j in range(NCORES):
        sl = slice(j * SL, (j + 1) * SL)
        in_maps.append(
            {
                "x_full": x,
                "x_sl": pad_rows(x[sl], SP),
                "norm_sl": pad_rows(norm[sl], SP),
                "normt_sl": pad_rows(norm_t[sl], SP),
                "wot": wot,
                "wbt": wbt,
                "ident": ident,
                "iota8": iota8,
                "gidx_o": gidx_o[j],
                "gidx_b": gidx_b[j],
                "sel_o": sel_o[j],
                "sel_b": sel_b[j],
            }
        )

    if return_graph:
        return nc, in_maps

    res = run(nc, in_maps, list(range(NCORES)))
    out = np.empty((cfg.N, cfg.C), dtype=np.float32)
    for j in range(NCORES):
        out[j * SL : (j + 1) * SL] = res.results[j]["out"][:SL]
    return out, res


def kernel(x, sources, targets, norm, norm_t, W_out, W_back):
    cfg = Cfg()
    out, _ = _kernel_impl(x, sources, targets, norm, norm_t, W_out, W_back, cfg)
    return out

